# revision 23
# baseline (speedup 1.0000x reference)
"""MoE combiner kernel for Trainium2 (8 NeuronCores, SPMD).

Computes out[i, d] = sum_e gates[i, e] * expert_outputs[e, d]
  gates:          [16384, 64]  fp32 (top-2 sparse rows, but dense contraction
                                     moves less HBM traffic than a gather)
  expert_outputs: [64, 4096]   fp32
  out:            [16384, 4096] fp32

Sharding: data-parallel over images. Each of the 8 cores computes a
[2048, 4096] slice of the output; the small expert table is replicated.

The correctness gate is rel_err < 2e-2, so full fp32 math is overkill:
inputs are rounded to fp16 on host, the PE does a single-pass K=64 fp16
matmul (fp32 PSUM accumulate), and the output is quantized to uint8 with
per-row scales during PSUM evacuation (dequantized on host). End-to-end
rel err ~1.2e-2, and the uint8 store quarters the dominant HBM write
traffic (8 MiB/core instead of 32 MiB).

(kernel_fp16_alt.py is the fp16-output variant: rel err ~4e-4 instead
of ~1.2e-2, but twice the HBM store traffic, so it degrades more under
co-tenant HBM contention.)

Pipeline notes (from trace analysis):
 - The PE clock gate (HAM) drops to 1.2 GHz after any ~1.7us idle gap
   and at this kernel's PE duty cycle never re-opens, nearly doubling
   matmul time. So the PE must never see a long gap: the warm-up bridges
   the input DMA, PSUM is 4 chunk-tiles deep so matmuls wait on the
   evacuation 4 chunks back (not the previous tile), and the whole
   output is staged in SBUF so store-DMA pacing never backpressures
   the PE/evacuation pipeline.
 - PSUM reads cap DVE/ACT at 1 elem/cycle/lane (fp32 src), so the
   fp16-converting evacuation is split across both engines, balanced by
   their cycle models.
"""

import numpy as np

NUM_EXPERTS = 64
NUM_IMAGES = 16384
D_MODEL = 4096
N_CORES = 8
ROWS = NUM_IMAGES // N_CORES  # 2048 images per core

IMG_TILE = 128          # images per matmul output tile (PSUM partition dim)
N_TILE = 512            # fp32 PSUM bank = 512 floats (max matmul N)
PS_W = 512              # PSUM chunk = 1 bank; 8 chunks per image tile.
                        # (1024-wide chunks amortize evac overhead better
                        # but their ~1.5us completion latency exceeds what
                        # a 3-deep PSUM pipeline can hide - measured worse.)
PS_BUFS = 7             # deep PSUM pipeline: matmul waits on the evacuation
                        # 7 chunks back (~2.4us of slack vs ~1.2us evac
                        # completion incl queueing)
DUMMY_AFTER = (2, 4, 6)  # chunks after which to insert a filler matmul:
                        # pads PE time per tile up to evacuation throughput
                        # so the PE never idles (any ~0.5us PE gap drops the
                        # clock gate 2.4->1.2GHz for the rest of the kernel).
                        # Fillers reuse the tile's stationary weights (rhs
                        # is zeros) so the weight-buffer pipelining that
                        # gives the 215ns/matmul pace is preserved.
OUT_BUFS = 8            # stage ALL output in SBUF (2 image tiles per buf)
WARM_MMS = 14           # HAM warm-up: the clock gate lifts only during
                        # an UNINTERRUPTED PE burst, and its free-running
                        # 4096-cycle window needs up to ~6.8us of gap-free
                        # streaming from the first matmul (once an early gap
                        # breaks the streak, the gate never opens and every
                        # matmul runs at 1.2 GHz). 14 cold matmuls = 5.98us
                        # of guaranteed streaming, and the staged input DMAs
                        # land ~1.7us before worst-case warm-up end, so the
                        # real matmuls continue the gap-free stream through
                        # the worst-case fire point. (10 warm-up matmuls
                        # measured ~0.5us faster but went cold 1 run in 6
                        # when a contended input DMA opened a pre-fire gap.)

_CACHE = {}


def _build_module():
    import concourse.bacc as bacc
    import concourse.mybir as mybir
    import concourse.tile as tile

    # Bacc (not bare Bass): its compile() pipeline runs
    # move_matmul_waits_to_ldweights + generate_event_semaphores, which
    # legalize multi-sem-wait instructions (the ISA allows one sync wait
    # per instruction; walrus rejects more).
    nc = bacc.Bacc("TRN2")
    f16 = mybir.dt.float16
    f32 = mybir.dt.float32

    n_img_tiles = ROWS // IMG_TILE          # 16

    with tile.TileContext(nc) as tc:
        with tc.tile_pool(name="dram", bufs=1, space="DRAM") as dram:
            # Packed input, ordered so one small leading DMA delivers
            # everything image tile 0 needs:
            #   [ gatesT tile0 (128) | E (4096) | gatesT tiles 1-15 (1920) ]
            allin = dram.tile([NUM_EXPERTS, ROWS + D_MODEL], f16,
                              kind="ExternalInput", name="allin",
                              uniquify=False)
            u8 = mybir.dt.uint8
            out = dram.tile([ROWS, D_MODEL], u8, kind="ExternalOutput",
                            name="out", uniquify=False)
            # out[t*128 + p, d] viewed as [p, t, d]: one DMA per image tile
            # covers 128 DRAM rows (8 KiB contiguous each) from one SBUF
            # tile spanning all 128 partitions.
            out_v = out.rearrange("(t p) d -> p t d", p=IMG_TILE)

            with tc.tile_pool(name="const", bufs=1) as cpool, \
                 tc.tile_pool(name="outp", bufs=OUT_BUFS) as outp, \
                 tc.tile_pool(name="psum", bufs=1,
                              space="PSUM") as pspool, \
                 tc.tile_pool(name="psum_dummy", bufs=1,
                              space="PSUM") as dummypool:
                in_sb = cpool.tile([NUM_EXPERTS, ROWS + D_MODEL], f16,
                                   name="in_sb")
                # Four input DMAs in dependency order, sized so each
                # lands just before the pipeline consumes it.
                cuts = [0, IMG_TILE + 1024, IMG_TILE + 2560,
                        IMG_TILE + D_MODEL, ROWS + D_MODEL]
                for a, b in zip(cuts[:-1], cuts[1:]):
                    nc.sync.dma_start(out=in_sb[:, a:b], in_=allin[:, a:b])
                e_sb = in_sb[:, IMG_TILE:IMG_TILE + D_MODEL]

                def gt_tile(it):
                    if it == 0:
                        return in_sb[:, :IMG_TILE]
                    base = IMG_TILE + D_MODEL + (it - 1) * IMG_TILE
                    return in_sb[:, base:base + IMG_TILE]

                # HAM warm-up, bridging until the first input DMA lands
                # (~9.5us): the clock gate lifts to 2.4 GHz after ~3.4us of
                # sustained PE activity, and the real matmuls then follow
                # with no >1.7us gap. The zero-fill runs on GPSIMD (idle
                # engine, short preamble; ACT has no memset) so the
                # PE starts ~2us earlier than with a DVE-side memset.
                warm_junk = cpool.tile([128, N_TILE], f16, name="warm_junk")
                nc.gpsimd.memset(warm_junk[:], 0)
                ps_dummy = dummypool.tile([128, N_TILE], f32,
                                          name="ps_dummy")
                # One manually-rotated 7-bank PSUM tile: consecutive
                # chunks land in address-adjacent banks, so most chunk
                # pairs evacuate as one 1024-wide instruction (amortizing
                # the fixed per-op cost) while PSUM is still released at
                # single-chunk granularity 7 slots deep.
                psbig = pspool.tile([128, PS_BUFS * PS_W], f32,
                                    name="psbig")
                for _ in range(WARM_MMS):
                    nc.tensor.matmul(ps_dummy[:],
                                     warm_junk[:, :IMG_TILE], warm_junk[:],
                                     start=True, stop=True)

                # Static greedy balance of PSUM evacuation between DVE and
                # ACT (fp32 PSUM src caps both at 1 elem/cycle/lane; the
                # measured per-512-chunk costs are ~equal at ~690ns, so
                # the greedy degenerates to a strict 64/64 alternation).
                dve_ns = 0.0
                act_ns = 0.0

                for it in range(n_img_tiles):
                    if it % 2 == 0:
                        ot = outp.tile([128, 2, D_MODEL], u8, name="ot")
                    lhsT = gt_tile(it)
                    pending = None
                    for half in range(D_MODEL // PS_W):
                        d0 = half * PS_W
                        k = it * (D_MODEL // PS_W) + half
                        slot = k % PS_BUFS
                        ps = psbig[:, slot * PS_W:(slot + 1) * PS_W]
                        nc.tensor.matmul(ps, lhsT, e_sb[:, d0:d0 + PS_W],
                                         start=True, stop=True)
                        if half in DUMMY_AFTER:
                            # Keep-warm filler (result never read).
                            nc.tensor.matmul(ps_dummy[:], lhsT,
                                             warm_junk[:NUM_EXPERTS, :],
                                             start=True, stop=True)
                        # Merge adjacent-slot pairs into one 1024-wide
                        # evacuation; singles when the rotation wraps.
                        todo = []
                        if half % 2 == 0:
                            if slot != PS_BUFS - 1:
                                pending = (slot, d0)
                            else:
                                todo.append((slot, d0, PS_W))
                        else:
                            if pending is not None:
                                todo.append((pending[0], pending[1],
                                             2 * PS_W))
                                pending = None
                            else:
                                todo.append((slot, d0, PS_W))
                        for eslot, ed0, width in todo:
                            src_ap = psbig[:, eslot * PS_W:
                                           eslot * PS_W + width]
                            dst = ot[:, it % 2, ed0:ed0 + width]
                            dcost = (148.0 + width) / 0.96
                            acost = (265.0 + width) / 1.2
                            if dve_ns + dcost <= act_ns + acost:
                                nc.vector.tensor_scalar_add(dst, src_ap,
                                                            128.5)
                                dve_ns += dcost
                            else:
                                nc.scalar.activation(
                                    dst, src_ap,
                                    mybir.ActivationFunctionType.Copy,
                                    bias=128.5)
                                act_ns += acost
                    if it == n_img_tiles - 1:
                        # Last tile: two 256 KiB stores so the final DMA
                        # (and its ~2us completion receipt) covers only
                        # half a tile after the last evacuation.
                        for a, b in ((0, D_MODEL // 2),
                                     (D_MODEL // 2, D_MODEL)):
                            nc.sync.dma_start(
                                out=out_v[:, it, a:b],
                                in_=ot[:, it % 2, a:b])
                    elif it == n_img_tiles - 2:
                        # Second-to-last tile: single-tile 512 KiB store.
                        nc.sync.dma_start(out=out_v[:, it:it + 1, :],
                                          in_=ot[:, it % 2:it % 2 + 1, :])
                    elif it % 2 == 1:
                        # One 1 MiB DMA per pair of image tiles.
                        nc.sync.dma_start(out=out_v[:, it - 1:it + 1, :],
                                          in_=ot[:])
    nc.compile()
    return nc


def _get_nc():
    if "nc" not in _CACHE:
        _CACHE["nc"] = _build_module()
    return _CACHE["nc"]


DEQUANT_C = 128.5       # matches round-to-nearest in the fp32->u8 convert
                        # (would be 128.0 if the convert truncated)


_SCALES = {}


def _make_in_maps(expert_outputs, gates):
    e16 = np.asarray(expert_outputs, dtype=np.float16)
    g32 = np.asarray(gates, dtype=np.float32)
    # Per-row quantization scale, folded into the gates so the matmul
    # emits pre-scaled outputs. Rigorous bound on the device value given
    # the fp16-rounded operands: bound_i = sum_e |g'[i,e]| * max_d |E[e,d]|
    # (gates are nonnegative), so u = x + 128.5 stays in (2, 255) - no
    # uint8 saturation.
    absmax = np.max(np.abs(e16.astype(np.float32)), axis=1)      # [64]
    bound = g32 @ absmax                                         # [16384]
    s32 = np.float32(126.0) / (bound * np.float32(1.001))
    g16 = (g32 * s32[:, None]).astype(np.float16)
    _SCALES["s"] = s32

    in_maps = []
    for c in range(N_CORES):
        rs = slice(c * ROWS, (c + 1) * ROWS)
        gt = g16[rs].T                      # [64, 2048]
        allin = np.ascontiguousarray(np.concatenate(
            [gt[:, :IMG_TILE], e16, gt[:, IMG_TILE:]], axis=1))
        in_maps.append({"allin": allin})
    return in_maps


def kernel(expert_outputs: np.ndarray, gates: np.ndarray) -> np.ndarray:
    from concourse.bass_utils import run_bass_kernel_spmd

    nc = _get_nc()
    in_maps = _make_in_maps(expert_outputs, gates)
    res = run_bass_kernel_spmd(nc, in_maps, core_ids=list(range(N_CORES)))
    u8 = np.concatenate([r["out"] for r in res.results], axis=0)
    inv_s = (1.0 / _SCALES["s"]).astype(np.float32)
    return (u8.astype(np.float32) - np.float32(DEQUANT_C)) * inv_s[:, None]


# revision 25
# speedup vs baseline: 2.8698x; 2.8698x over previous
"""MoE combiner kernel for Trainium2 (8 NeuronCores, SPMD).

Computes out[i, d] = sum_e gates[i, e] * expert_outputs[e, d]
  gates:          [16384, 64]  fp32 (top-2 sparse rows, but dense contraction
                                     moves less HBM traffic than a gather)
  expert_outputs: [64, 4096]   fp32
  out:            [16384, 4096] fp32

Sharding: data-parallel over images. Each of the 8 cores computes a
[2048, 4096] slice of the output; the small expert table is replicated.

The correctness gate is rel_err < 2e-2, so full fp32 math is overkill:
inputs are rounded to fp16 on host, the PE does a single-pass K=64 fp16
matmul (fp32 PSUM accumulate), and the output is quantized to uint8 with
per-row scales during PSUM evacuation (dequantized on host). End-to-end
rel err ~1.2e-2, and the uint8 store quarters the dominant HBM write
traffic (8 MiB/core instead of 32 MiB).

(kernel_fp16_alt.py is the fp16-output variant: rel err ~4e-4 instead
of ~1.2e-2, but twice the HBM store traffic, so it degrades more under
co-tenant HBM contention.)

Pipeline notes (from trace analysis):
 - The PE clock gate (HAM) drops to 1.2 GHz after any ~1.7us idle gap
   and at this kernel's PE duty cycle never re-opens, nearly doubling
   matmul time. So the PE must never see a long gap: the warm-up bridges
   the input DMA, PSUM is 4 chunk-tiles deep so matmuls wait on the
   evacuation 4 chunks back (not the previous tile), and the whole
   output is staged in SBUF so store-DMA pacing never backpressures
   the PE/evacuation pipeline.
 - PSUM reads cap DVE/ACT at 1 elem/cycle/lane (fp32 src), so the
   fp16-converting evacuation is split across both engines, balanced by
   their cycle models.
"""

import numpy as np

NUM_EXPERTS = 64
NUM_IMAGES = 16384
D_MODEL = 4096
N_CORES = 8
ROWS = NUM_IMAGES // N_CORES  # 2048 images per core

IMG_TILE = 128          # images per matmul output tile (PSUM partition dim)
N_TILE = 512            # fp32 PSUM bank = 512 floats (max matmul N)
PS_W = 512              # PSUM chunk = 1 bank; 8 chunks per image tile.
                        # (1024-wide chunks amortize evac overhead better
                        # but their ~1.5us completion latency exceeds what
                        # a 3-deep PSUM pipeline can hide - measured worse.)
PS_BUFS = 7             # deep PSUM pipeline: matmul waits on the evacuation
                        # 7 chunks back (~2.4us of slack vs ~1.2us evac
                        # completion incl queueing)
DUMMY_AFTER = (2, 4, 6)  # chunks after which to insert a filler matmul:
                        # pads PE time per tile up to evacuation throughput
                        # so the PE never idles (any ~0.5us PE gap drops the
                        # clock gate 2.4->1.2GHz for the rest of the kernel).
                        # Fillers reuse the tile's stationary weights (rhs
                        # is zeros) so the weight-buffer pipelining that
                        # gives the 215ns/matmul pace is preserved.
OUT_BUFS = 8            # stage ALL output in SBUF (2 image tiles per buf)
WARM_MMS = 13           # HAM warm-up: the clock gate lifts only during
                        # an UNINTERRUPTED PE burst, and its free-running
                        # 4096-cycle window needs up to ~6.8us of gap-free
                        # streaming from the first matmul (once an early gap
                        # breaks the streak, the gate never opens and every
                        # matmul runs at 1.2 GHz). 14 cold matmuls = 5.98us
                        # of guaranteed streaming, and the staged input DMAs
                        # land ~1.7us before worst-case warm-up end, so the
                        # real matmuls continue the gap-free stream through
                        # the worst-case fire point. (10 warm-up matmuls
                        # measured ~0.5us faster but went cold 1 run in 6
                        # when a contended input DMA opened a pre-fire gap.)

_CACHE = {}


def _build_module():
    import concourse.bacc as bacc
    import concourse.mybir as mybir
    import concourse.tile as tile

    # Bacc (not bare Bass): its compile() pipeline runs
    # move_matmul_waits_to_ldweights + generate_event_semaphores, which
    # legalize multi-sem-wait instructions (the ISA allows one sync wait
    # per instruction; walrus rejects more).
    nc = bacc.Bacc("TRN2")
    f16 = mybir.dt.float16
    f32 = mybir.dt.float32

    n_img_tiles = ROWS // IMG_TILE          # 16

    with tile.TileContext(nc) as tc:
        with tc.tile_pool(name="dram", bufs=1, space="DRAM") as dram:
            # Packed input, ordered so one small leading DMA delivers
            # everything image tile 0 needs:
            #   [ gatesT tile0 (128) | E (4096) | gatesT tiles 1-15 (1920) ]
            allin = dram.tile([NUM_EXPERTS, ROWS + D_MODEL], f16,
                              kind="ExternalInput", name="allin",
                              uniquify=False)
            u8 = mybir.dt.uint8
            out = dram.tile([ROWS, D_MODEL], u8, kind="ExternalOutput",
                            name="out", uniquify=False)
            # out[t*128 + p, d] viewed as [p, t, d]: one DMA per image tile
            # covers 128 DRAM rows (8 KiB contiguous each) from one SBUF
            # tile spanning all 128 partitions.
            out_v = out.rearrange("(t p) d -> p t d", p=IMG_TILE)

            with tc.tile_pool(name="const", bufs=1) as cpool, \
                 tc.tile_pool(name="outp", bufs=OUT_BUFS) as outp, \
                 tc.tile_pool(name="psum", bufs=PS_BUFS,
                              space="PSUM") as pspool, \
                 tc.tile_pool(name="psum_dummy", bufs=1,
                              space="PSUM") as dummypool:
                in_sb = cpool.tile([NUM_EXPERTS, ROWS + D_MODEL], f16,
                                   name="in_sb")
                # Four input DMAs in dependency order, sized so each
                # lands just before the pipeline consumes it.
                cuts = [0, IMG_TILE + 1024, IMG_TILE + 2560,
                        IMG_TILE + D_MODEL, ROWS + D_MODEL]
                for a, b in zip(cuts[:-1], cuts[1:]):
                    nc.sync.dma_start(out=in_sb[:, a:b], in_=allin[:, a:b])
                e_sb = in_sb[:, IMG_TILE:IMG_TILE + D_MODEL]

                def gt_tile(it):
                    if it == 0:
                        return in_sb[:, :IMG_TILE]
                    base = IMG_TILE + D_MODEL + (it - 1) * IMG_TILE
                    return in_sb[:, base:base + IMG_TILE]

                # HAM warm-up, bridging until the first input DMA lands
                # (~9.5us): the clock gate lifts to 2.4 GHz after ~3.4us of
                # sustained PE activity, and the real matmuls then follow
                # with no >1.7us gap. The zero-fill runs on GPSIMD (idle
                # engine, short preamble; ACT has no memset) so the
                # PE starts ~2us earlier than with a DVE-side memset.
                warm_junk = cpool.tile([128, N_TILE], f16, name="warm_junk")
                nc.gpsimd.memset(warm_junk[:], 0)
                ps_dummy = dummypool.tile([128, N_TILE], f32,
                                          name="ps_dummy")
                for _ in range(WARM_MMS):
                    nc.tensor.matmul(ps_dummy[:],
                                     warm_junk[:, :IMG_TILE], warm_junk[:],
                                     start=True, stop=True)

                # Static greedy balance of PSUM evacuation between DVE and
                # ACT (fp32 PSUM src caps both at 1 elem/cycle/lane; the
                # measured per-512-chunk costs are ~equal at ~690ns, so
                # the greedy degenerates to a strict 64/64 alternation).
                dve_ns = 0.0
                act_ns = 0.0

                for it in range(n_img_tiles):
                    if it % 2 == 0:
                        ot = outp.tile([128, 2, D_MODEL], u8, name="ot")
                    lhsT = gt_tile(it)
                    for half in range(D_MODEL // PS_W):
                        d0 = half * PS_W
                        ps = pspool.tile([128, PS_W], f32, name="ps")
                        nc.tensor.matmul(ps[:], lhsT, e_sb[:, d0:d0 + PS_W],
                                         start=True, stop=True)
                        if half in DUMMY_AFTER and it < n_img_tiles - 1:
                            # Keep-warm filler (result never read); the
                            # final tile skips it - nothing left to keep
                            # warm, and fillers sit ahead of the last
                            # chunks in the PE queue, delaying the final
                            # evacuation and store by ~0.6us.
                            nc.tensor.matmul(ps_dummy[:], lhsT,
                                             warm_junk[:NUM_EXPERTS, :],
                                             start=True, stop=True)
                        # Evacuate + quantize on whichever engine is less
                        # loaded. The per-row scale is folded into the
                        # gates host-side, so this is just u = x + 128.5
                        # with a rounding uint8 convert - plain-copy speed.
                        dst = ot[:, it % 2, d0:d0 + PS_W]
                        if dve_ns <= act_ns:
                            nc.vector.tensor_scalar_add(dst, ps[:], 128.5)
                            dve_ns += 690.0
                        else:
                            nc.scalar.activation(
                                dst, ps[:],
                                mybir.ActivationFunctionType.Copy,
                                bias=128.5)
                            act_ns += 690.0
                    if it == n_img_tiles - 1:
                        # Last tile: two 256 KiB stores so the final DMA
                        # (and its ~2us completion receipt) covers only
                        # half a tile after the last evacuation.
                        for a, b in ((0, D_MODEL // 2),
                                     (D_MODEL // 2, D_MODEL)):
                            nc.sync.dma_start(
                                out=out_v[:, it, a:b],
                                in_=ot[:, it % 2, a:b])
                    elif it == n_img_tiles - 2:
                        # Second-to-last tile: single-tile 512 KiB store.
                        nc.sync.dma_start(out=out_v[:, it:it + 1, :],
                                          in_=ot[:, it % 2:it % 2 + 1, :])
                    elif it % 2 == 1:
                        # One 1 MiB DMA per pair of image tiles.
                        nc.sync.dma_start(out=out_v[:, it - 1:it + 1, :],
                                          in_=ot[:])
    nc.compile()
    return nc


def _get_nc():
    if "nc" not in _CACHE:
        _CACHE["nc"] = _build_module()
    return _CACHE["nc"]


DEQUANT_C = 128.5       # matches round-to-nearest in the fp32->u8 convert
                        # (would be 128.0 if the convert truncated)


_SCALES = {}


def _make_in_maps(expert_outputs, gates):
    e16 = np.asarray(expert_outputs, dtype=np.float16)
    g32 = np.asarray(gates, dtype=np.float32)
    # Per-row quantization scale, folded into the gates so the matmul
    # emits pre-scaled outputs. Rigorous bound on the device value given
    # the fp16-rounded operands: bound_i = sum_e |g'[i,e]| * max_d |E[e,d]|
    # (gates are nonnegative), so u = x + 128.5 stays in (2, 255) - no
    # uint8 saturation.
    absmax = np.max(np.abs(e16.astype(np.float32)), axis=1)      # [64]
    bound = g32 @ absmax                                         # [16384]
    s32 = np.float32(126.0) / (bound * np.float32(1.001))
    g16 = (g32 * s32[:, None]).astype(np.float16)
    _SCALES["s"] = s32

    in_maps = []
    for c in range(N_CORES):
        rs = slice(c * ROWS, (c + 1) * ROWS)
        gt = g16[rs].T                      # [64, 2048]
        allin = np.ascontiguousarray(np.concatenate(
            [gt[:, :IMG_TILE], e16, gt[:, IMG_TILE:]], axis=1))
        in_maps.append({"allin": allin})
    return in_maps


def kernel(expert_outputs: np.ndarray, gates: np.ndarray) -> np.ndarray:
    from concourse.bass_utils import run_bass_kernel_spmd

    nc = _get_nc()
    in_maps = _make_in_maps(expert_outputs, gates)
    res = run_bass_kernel_spmd(nc, in_maps, core_ids=list(range(N_CORES)))
    u8 = np.concatenate([r["out"] for r in res.results], axis=0)
    inv_s = (1.0 / _SCALES["s"]).astype(np.float32)
    return (u8.astype(np.float32) - np.float32(DEQUANT_C)) * inv_s[:, None]
